# revision 32
# baseline (speedup 1.0000x reference)
"""Pairwise squared-euclidean distance kernel for Trainium2 (8 NeuronCores).

z[i, j] = ||x_i||^2 + ||y_j||^2 - 2 * <x_i, y_j>

Sharding: x rows split across 8 cores (1024 rows each), y replicated.
Each core computes a [1024, 8192] tile of the output with no communication.

Per-core algorithm (fp32 data; cross terms in fp16 on the PE), measured
~135.5 us HW exec (vs ~116 us HBM roofline for the 41.4 MB/core of
compulsory traffic + ~18 us fixed NRT preamble/drain/postamble):
  1. Load x shard, xsq row norms (ScalarE Square+accum), PE-transpose x
     casting to fp16 and folding the -2 scale during PSUM evacuation.
  2. Single n-outer pipeline over 16 chunks of 512 y rows, with the
     transpose stage running one chunk ahead of the matmul stage:
     a. ALL of y stays staged in SBUF (64 KB/partition), loaded by 4
        up-front HWDGE DMAs of growing size (1+1+2+4 MB) on the ScalarE
        ring. In-loop dma_start issues would sit behind engine ops in
        the sequencer FIFO and cap prefetch at ~4 chunks, idling the
        DMA engines during fill; staged bulk loads issue in ~3 us and
        each lands well before its chunks are consumed.
     b. PE-transpose (fp32) into yT fp16 (VectorE cast evac); ScalarE
        squares the same transpose PSUM into yTs/yTsq1 fp16; GpSimd
        (otherwise idle; can't touch PSUM) sums the two 128-row halves
        into yTs in place so one matmul covers the full 256-dim ysq
        sum.
     c. Per m-tile PSUM bank: two fp16 cross-term matmuls (stationary
        xT slices) then ones128^T @ yTs, which adds ||y_j||^2 broadcast
        to all partitions -- no elementwise ysq add anywhere. ysq last:
        yTs is the latest-ready input.
     d. Evacuate adding xsq as per-partition bias (ScalarE bias path /
        VectorE tensor_scalar_add alternating) into one [128, 8, 512]
        tile; ONE 2 MB out-DMA per chunk via a scattered DRAM view
        (each dma_start costs the issuing sequencer ~700 ns, so 8
        stripe-DMAs/chunk would eat the Sync sequencer).

Known-good environment notes: tensor_tensor_reduce crashes the device
(NRT_EXEC_UNIT_UNRECOVERABLE) - do not use. fp32r matmuls never warm
the HAM clock gate and self-load weights serially (~536ns/mm).
GpSimd SBUF tensor_tensor runs ~53 G elem/s (3x slower than docs
claim). Engine clocks vary ~20% run-to-run (P0 downclock): compare
runs via the constant-work ACT_TABLE_LOAD slice (1283 ns full clock).
"""

import os

import numpy as np

import concourse.bacc as bacc
import concourse.mybir as mybir
import concourse.tile as tile
from concourse.bass_utils import run_bass_kernel_spmd
from concourse.masks import make_identity

N_CORES = 8
N_FULL = 8192  # total x rows
M_Y = 8192  # y rows
D = 256  # feature dim
N_SHARD = N_FULL // N_CORES  # 1024 x rows per core

P = 128
NT = 512  # matmul free-dim tile (one fp32 PSUM bank)
FP32 = mybir.dt.float32
FP16 = mybir.dt.float16
AF = mybir.ActivationFunctionType
ALU = mybir.AluOpType

_CACHE = {}
LAST_RESULTS = None


def _build():
    nc = bacc.Bacc("TRN2", target_bir_lowering=False, debug=False, num_devices=N_CORES)
    x_d = nc.dram_tensor("x", [N_SHARD, D], FP32, kind="ExternalInput").ap()
    y_d = nc.dram_tensor("y", [M_Y, D], FP32, kind="ExternalInput").ap()
    out_d = nc.dram_tensor("out", [N_SHARD, M_Y], FP32, kind="ExternalOutput").ap()

    M_TILES = N_SHARD // P  # 8 m-tiles (x rows)
    N_CHUNKS = M_Y // NT  # 16 chunks of 512 y rows / output cols

    with tile.TileContext(nc) as tc:
        with (
            tc.tile_pool(name="const", bufs=1) as const,
            tc.tile_pool(name="sqx", bufs=1) as sqx,
            tc.tile_pool(name="sq", bufs=4) as sqp,
            tc.tile_pool(name="outp", bufs=3) as outp,
            tc.tile_pool(name="psmm", bufs=8, space="PSUM") as psmm,
        ):
            identity = const.tile([P, P], FP32)
            make_identity(nc, identity)
            ones128 = const.tile([P, P], FP16)
            nc.gpsimd.memset(ones128[:], 1.0)

            xsq = const.tile([P, M_TILES], FP32)
            x_nat = const.tile([P, M_TILES, D], FP32)
            xT = [
                const.tile([P, N_SHARD], FP16, tag=f"xT{c}", name=f"xT{c}")
                for c in range(2)
            ]
            yT = [
                const.tile([P, M_Y], FP16, tag=f"yT{c}", name=f"yT{c}")
                for c in range(2)
            ]
            yTsq1 = const.tile([P, M_Y], FP16, tag="yTsq1", name="yTsq1")
            # yTs = square(yT half0) + square(yT half1), summed elementwise
            # by GpSimd (otherwise idle): one ones128^T @ yTs matmul then
            # yields the full 256-dim ysq sum.
            yTs = const.tile([P, M_Y], FP16, tag="yTs", name="yTs")

            # ---- main pipeline: per 512-row / 512-col y chunk ----
            # ALL of y stays staged in SBUF (64 KB/partition), loaded by 4
            # up-front HWDGE DMAs of growing size. dma_start issues embedded
            # mid-stream sit behind engine ops in the sequencer FIFO and cap
            # prefetch at ~4 chunks (leaving the DMA engines idle during
            # fill); a handful of staged bulk loads issue in ~3 us total and
            # each lands well before its chunks are consumed.
            ylin = sqx.tile([P, N_CHUNKS, 4, D], FP32, name="ylin")
            for lo, hi in ((0, 2), (2, 4), (4, 8), (8, 16)):
                nc.scalar.dma_start(
                    ylin[:, lo:hi],
                    y_d[lo * NT : hi * NT, :].rearrange(
                        "(n t p) d -> p n t d", p=P, t=4
                    ),
                )

            # ---- x: load, row norms, transpose (x -2 folded into evac) ----
            # x PSUM tiles are evacuated (and their slots released) while
            # the first y chunks are still loading.
            nc.sync.dma_start(x_nat[:], x_d.rearrange("(t p) d -> p t d", p=P))
            for t in range(M_TILES):
                sq = sqp.tile([P, D], FP32, tag="sq")
                nc.scalar.activation(
                    sq[:], x_nat[:, t, :], AF.Square, accum_out=xsq[:, t : t + 1]
                )
            for c in range(2):
                for h in range(2):
                    ps = psmm.tile([P, NT], FP32, tag="mm")
                    for s in range(4):
                        t = h * 4 + s
                        nc.tensor.transpose(
                            ps[:, s * P : (s + 1) * P],
                            x_nat[:, t, c * P : (c + 1) * P],
                            identity,
                        )
                    nc.vector.tensor_scalar_mul(
                        xT[c][:, h * NT : (h + 1) * NT], ps[:], -2.0
                    )

            # The transpose stage runs one chunk ahead of the matmul stage
            # so the PE fills its wait-for-evac gap with the next chunk's
            # transposes and the yT/yTs evacs overlap the matmul block.
            def load_transpose_chunk(n):
                nsl = slice(n * NT, (n + 1) * NT)
                for c in range(2):
                    ps = psmm.tile([P, NT], FP32, tag="mm")
                    for s in range(4):
                        nc.tensor.transpose(
                            ps[:, s * P : (s + 1) * P],
                            ylin[:, n, s, c * P : (c + 1) * P],
                            identity,
                        )
                    nc.vector.tensor_copy(yT[c][:, nsl], ps[:])
                    nc.scalar.activation(
                        (yTs if c == 0 else yTsq1)[:, nsl], ps[:], AF.Square
                    )
                nc.gpsimd.tensor_tensor(
                    yTs[:, nsl], yTs[:, nsl], yTsq1[:, nsl], ALU.add
                )

            load_transpose_chunk(0)
            for n in range(N_CHUNKS):
                nsl = slice(n * NT, (n + 1) * NT)
                if n + 1 < N_CHUNKS:
                    load_transpose_chunk(n + 1)

                pms = [
                    psmm.tile([P, NT], FP32, tag="mm", name=f"pm_{n}_{m}")
                    for m in range(M_TILES)
                ]
                # ysq matmul last: the GpSimd-produced yTs is the latest
                # input in the per-chunk dependency chain, so banks start
                # on the cross terms (which only need yT) first.
                for m in range(M_TILES):
                    msl = slice(m * P, (m + 1) * P)
                    nc.tensor.matmul(
                        pms[m][:], xT[0][:, msl], yT[0][:, nsl],
                        start=True, stop=False,
                    )
                    nc.tensor.matmul(
                        pms[m][:], xT[1][:, msl], yT[1][:, nsl],
                        start=False, stop=False,
                    )
                    nc.tensor.matmul(
                        pms[m][:], ones128[:], yTs[:, nsl],
                        start=False, stop=True,
                    )
                # one [128, 8, 512] tile per chunk -> a single 2 MB out DMA:
                # each dma_start costs the issuing sequencer ~700 ns
                # (DIRECT2D), so 8 stripes/chunk would eat most of the Sync
                # sequencer; merged, it is one issue per chunk.
                ot = outp.tile([P, M_TILES, NT], FP32, tag="ot")
                for m in range(M_TILES):
                    if m % 2 == 0:
                        nc.scalar.activation(
                            ot[:, m, :], pms[m][:], AF.Identity,
                            bias=xsq[:, m : m + 1], scale=1.0,
                        )
                    else:
                        nc.vector.tensor_scalar_add(
                            ot[:, m, :], pms[m][:], xsq[:, m : m + 1]
                        )
                nc.sync.dma_start(
                    out_d[:, n * NT : (n + 1) * NT].rearrange(
                        "(m p) j -> p m j", p=P
                    ),
                    ot[:],
                )

    nc.compile()
    return nc


def _get_nc():
    if "nc" not in _CACHE:
        _CACHE["nc"] = _build()
    return _CACHE["nc"]


def kernel(x: np.ndarray, y: np.ndarray) -> np.ndarray:
    global LAST_RESULTS
    x = np.ascontiguousarray(np.asarray(x, dtype=np.float32))
    y = np.ascontiguousarray(np.asarray(y, dtype=np.float32))
    assert x.shape == (N_FULL, D) and y.shape == (M_Y, D)

    nc = _get_nc()
    in_maps = [
        {"x": x[i * N_SHARD : (i + 1) * N_SHARD], "y": y} for i in range(N_CORES)
    ]
    res = run_bass_kernel_spmd(
        nc,
        in_maps,
        core_ids=list(range(N_CORES)),
        trace=bool(os.environ.get("BASS_KERNEL_TRACE")),
    )
    LAST_RESULTS = res
    return np.concatenate([res.results[i]["out"] for i in range(N_CORES)], axis=0)
